# revision 28
# baseline (speedup 1.0000x reference)
"""Affine coupling transform (RealNVP-style) on 8 Trainium2 NeuronCores.

Data parallel: batch dim sharded 8 ways, weights replicated. Per core:
  x_shard [4096, 1024] -> out [4096, 1024], logabsdet [4096]

Math (per row):
  x_id = x[even cols], x_tr = x[odd cols]
  h = relu(x_id @ W1 + b1); params = h @ W2 + b2
  scale = sigmoid(params[:, :512] + 2) + 1e-3; shift = params[:, 512:]
  out[even] = x_id; out[odd] = x_tr * scale + shift
  logabsdet = sum(log(scale))

Kernel structure per core (b-tiles of 512 rows):
  - PE transposes x_id chunks to feature-major (fp32 DMA transpose unsupported)
  - both matmuls in float32r (full PE rate at N=512, fp32 data)
  - logabsdet via running product: prod((scale)*C) on DVE, single Ln at the
    end -- avoids per-tile ACT table swaps (sigmoid and ln live in different
    ACT table sets; each swap costs ~2.7us)
"""

import sys

if "/opt/trn_rl_repo" not in sys.path:
    sys.path.insert(0, "/opt/trn_rl_repo")

import math

import numpy as np

import concourse.bass as bass
import concourse.mybir as mybir
import concourse.tile as tile
from concourse import bacc
from concourse.bass_utils import run_bass_kernel_spmd
from concourse.masks import make_identity

N_CORES = 8
B_FULL = 32768
F_TOTAL = 1024
F_HALF = 512
H = 512
B = B_FULL // N_CORES  # rows per core
P = 128
BT = 512               # rows per b-tile (matmul moving dim)
NBT = B // BT          # b-tiles per core
NCH = BT // P          # 128-row chunks per b-tile
FC = F_HALF // P       # feature chunks (contraction of MM1)
JC = H // P            # hidden chunks (contraction of MM2)
SCALE_EPS = 1e-3
# The full 512-factor product of scales sits near exp(-77+-13); rescale by
# exp(+RESCALE_LOG) at the last tree level so the Ln input is centered near
# 1.0 (the HW Ln spline mishandles inputs far below 1).
RESCALE_LOG = 80.0
RESCALE_K = float(np.exp(RESCALE_LOG, dtype=np.float64))
# matmul operand dtype: "float16" (1 cyc/row on PE) or "float32r"
# (11-bit-mantissa fp32, but 2 cyc/row and serialized weight loads)
MM_DTYPE = "float16"
# "pe": tensor-engine transpose via identity matmul. ("dma" XBAR transpose
# hard-crashes the device: this Tile version has no xbar_mode hazard
# serialization, hitting the known DMATranspose/DMACopy HW hang.)
TRANSPOSE_MODE = "pe"

TRACE = False
TRACE_DIR = None
LAST_RESULTS = None

_cache = {}


def _round_fp32r(a: np.ndarray) -> np.ndarray:
    """Round fp32 to the fp32r memory format: mantissa rounded (RNE) to 11
    explicit bits, low 12 bits zero."""
    bits = np.ascontiguousarray(a, dtype=np.float32).view(np.uint32)
    keep = np.uint32(0xFFFFF000)
    half = np.uint32(0x7FF)
    lsb = (bits >> np.uint32(12)) & np.uint32(1)
    rounded = (bits + half + lsb) & keep
    return rounded.view(np.float32)

f32 = mybir.dt.float32
f32r = mybir.dt.float32r


def _build(with_b2: bool, mm_dtype=None) -> bass.Bass:
    fmm = {"float16": mybir.dt.float16, "float32r": f32r}[mm_dtype or MM_DTYPE]
    dma_tp = TRANSPOSE_MODE == "dma" and fmm == mybir.dt.float16
    nc = bacc.Bacc(None)

    x = nc.dram_tensor("x", [B, F_TOTAL], f32, kind="ExternalInput")
    w1 = nc.dram_tensor("w1", [F_HALF, H], fmm, kind="ExternalInput")
    b1 = nc.dram_tensor("b1", [H], f32, kind="ExternalInput")
    w2 = nc.dram_tensor("w2", [H, 2 * F_HALF], fmm, kind="ExternalInput")
    b2 = nc.dram_tensor("b2", [2 * F_HALF], f32, kind="ExternalInput")
    out = nc.dram_tensor("out", [B, F_TOTAL], f32, kind="ExternalOutput")
    logdet = nc.dram_tensor("logdet", [B], f32, kind="ExternalOutput")

    Mult = mybir.AluOpType.mult
    Add = mybir.AluOpType.add
    AF = mybir.ActivationFunctionType

    with tile.TileContext(nc) as tc:
        with (
            tc.tile_pool(name="singles", bufs=1) as singles,
            tc.tile_pool(name="xin", bufs=5) as xin,
            tc.tile_pool(name="xT", bufs=3) as xTp,
            tc.tile_pool(name="hT", bufs=3) as hTp,
            tc.tile_pool(name="eps", bufs=4) as epsp,
            tc.tile_pool(name="xc", bufs=3) as xcp,
            tc.tile_pool(name="tp_ps", bufs=(1 if dma_tp else 3), space="PSUM") as tp_ps,
            tc.tile_pool(name="h_ps", bufs=(3 if dma_tp else 2), space="PSUM") as h_ps,
            tc.tile_pool(name="p2_ps", bufs=(4 if dma_tp else 3), space="PSUM") as p2_ps,
        ):
            # ---- constants ----
            ident = singles.tile([P, P], f32)
            make_identity(nc, ident)

            w1s = singles.tile([P, FC, H], fmm)
            w2s = singles.tile([P, JC, 2 * F_HALF], fmm)
            b1s = singles.tile([P, JC], f32)
            nc.sync.dma_start(b1s, b1.rearrange("(jo ji) -> ji jo", ji=P))
            if with_b2:
                b2s = singles.tile([P, 2 * F_HALF], f32)
                nc.sync.dma_start(
                    b2s, b2[None, :].broadcast_to([P, 2 * F_HALF])
                )
            two_col = singles.tile([P, 1], f32)
            nc.vector.memset(two_col, 2.0)

            # per-row partial products of scale*C: block t*NCH+c holds 128
            # 4-factor partials for rows [ (t*NCH+c)*128 , +128 )
            prodacc = singles.tile([P, NBT * NCH, P // 2], f32)

            def load_tile(t):
                r0 = t * BT
                xb = xin.tile([P, NCH, F_TOTAL], f32, tag="xt")
                if t == 0:
                    # per-chunk DMAs so the first transpose isn't gated on
                    # the whole 2MB tile landing
                    for c in range(NCH):
                        nc.sync.dma_start(
                            xb[:, c, :], x[r0 + c * P : r0 + (c + 1) * P, :]
                        )
                else:
                    nc.sync.dma_start(
                        xb,
                        x[r0 : r0 + BT, :].rearrange("(c p) f -> p c f", p=P),
                    )
                return xb, [xb[:, c, :] for c in range(NCH)]

            def transpose_tile(xts):
                xT = xTp.tile([P, FC, BT], fmm, tag="xT")
                for c in range(NCH):
                    xtr = xts[c].rearrange("p (f two) -> p f two", two=2)
                    if dma_tp:
                        # cast even cols to fp16, then XBAR-transpose via DMA
                        xc = xcp.tile([P, F_HALF], fmm, tag="xc")
                        nc.vector.tensor_copy(out=xc, in_=xtr[:, :, 0])
                        for fc in range(FC):
                            nc.sync.dma_start_transpose(
                                xT[:, fc, c * P : (c + 1) * P],
                                xc[:, fc * P : (fc + 1) * P],
                            )
                    else:
                        for fc in range(FC):
                            tps = tp_ps.tile([P, P], f32, tag="tp")
                            nc.tensor.transpose(
                                tps, xtr[:, fc * P : (fc + 1) * P, 0], ident
                            )
                            nc.vector.tensor_copy(
                                out=xT[:, fc, c * P : (c + 1) * P], in_=tps
                            )
                return xT

            # software-pipelined emission: the next tile's transposes (PE) and
            # their PSUM->SBUF copies (DVE) are queued BEFORE this tile's
            # MM2/epilogue, so the strict-FIFO DVE queue doesn't head-of-line
            # block the next tile's transposes behind the epilogue.
            xb, xts = load_tile(0)
            nc.sync.dma_start(w1s, w1.rearrange("(fo fi) j -> fi fo j", fi=P))
            xT = transpose_tile(xts)
            nc.sync.dma_start(w2s, w2.rearrange("(jo ji) p -> ji jo p", ji=P))
            next_xts = None
            next_xT = None
            for t in range(NBT):
                r0 = t * BT
                if t + 1 < NBT:
                    next_xb, next_xts = load_tile(t + 1)

                # ---- MM1: hT[j, b] = relu(sum_f W1[f,j] xT[f,b] + b1[j]) ----
                hT = hTp.tile([P, JC, BT], fmm, tag="hT")
                for jc in range(JC):
                    hps = h_ps.tile([P, BT], f32, tag="h")
                    for fc in range(FC):
                        nc.tensor.matmul(
                            hps,
                            w1s[:, fc, jc * P : (jc + 1) * P],
                            xT[:, fc, :],
                            start=(fc == 0),
                            stop=(fc == FC - 1),
                        )
                    nc.scalar.activation(
                        out=hT[:, jc, :],
                        in_=hps,
                        func=AF.Relu,
                        bias=b1s[:, jc : jc + 1],
                    )

                if t + 1 < NBT:
                    next_xT = transpose_tile(next_xts)

                # ---- MM2 + coupling epilogue, per 128-row chunk ----
                for c in range(NCH):
                    pps = []
                    for ph in range(2):
                        pp = p2_ps.tile([P, F_HALF], f32, tag="p2")
                        for jc in range(JC):
                            nc.tensor.matmul(
                                pp,
                                hT[:, jc, c * P : (c + 1) * P],
                                w2s[:, jc, ph * F_HALF : (ph + 1) * F_HALF],
                                start=(jc == 0),
                                stop=(jc == JC - 1),
                            )
                        pps.append(pp)

                    if with_b2:
                        nc.vector.tensor_add(
                            out=pps[0], in0=pps[0], in1=b2s[:, :F_HALF]
                        )
                        nc.vector.tensor_add(
                            out=pps[1], in0=pps[1], in1=b2s[:, F_HALF:]
                        )

                    # scale path: sigmoid(u + 2), then cs = (sig + eps)*C;
                    # two pairwise-product halvings leave 128 partials per
                    # row in prodacc (reduced to one product at the end)
                    sig = epsp.tile([P, F_HALF], f32, tag="sig")
                    nc.scalar.activation(
                        out=sig, in_=pps[0], func=AF.Sigmoid, bias=two_col
                    )
                    cs = epsp.tile([P, F_HALF], f32, tag="cs")
                    nc.vector.tensor_scalar_add(cs, sig, SCALE_EPS)
                    idx = t * NCH + c
                    q1 = epsp.tile([P, F_HALF // 2], f32, tag="q1")
                    nc.vector.tensor_mul(
                        out=q1, in0=cs[:, : F_HALF // 2], in1=cs[:, F_HALF // 2 :]
                    )
                    q2 = epsp.tile([P, F_HALF // 4], f32, tag="q2")
                    nc.vector.tensor_mul(out=q2, in0=q1[:, :P], in1=q1[:, P:])
                    nc.vector.tensor_mul(
                        out=prodacc[:, idx, :],
                        in0=q2[:, : P // 2],
                        in1=q2[:, P // 2 :],
                    )

                    # out[odd] = x_tr * (cs/C) + shift ; out[even] = x_id
                    # out[even] = x[even] for free: x was loaded into this
                    # buffer and only the odd columns are overwritten
                    xtr = xts[c].rearrange("p (f two) -> p f two", two=2)
                    tmp = epsp.tile([P, F_HALF], f32, tag="tmp")
                    nc.vector.tensor_mul(out=tmp, in0=xtr[:, :, 1], in1=cs)
                    nc.vector.tensor_add(out=xtr[:, :, 1], in0=tmp, in1=pps[1])

                    if t == NBT - 1:
                        nc.sync.dma_start(
                            out[r0 + c * P : r0 + (c + 1) * P, :], xts[c]
                        )
                    elif c == NCH - 1:
                        nc.sync.dma_start(
                            out[r0 : r0 + BT, :].rearrange(
                                "(c p) f -> p c f", p=P
                            ),
                            xb,
                        )

                xb = next_xb
                xts = next_xts
                xT = next_xT

            # ---- reduce the 128 partials per block to one product via an
            # in-block pairwise tree, then logabsdet = ln(prod) - 512*ln(C),
            # transposed for a contiguous DRAM write ----
            NB = NBT * NCH
            tree = prodacc
            w = P // 2
            while w > 1:
                nxt = singles.tile([P, NB, w // 2], f32, tag=f"tree{w}")
                if w == 2:
                    # last level: fold in exp(+RESCALE_LOG) to keep the full
                    # 512-factor product inside fp32 range
                    nc.vector.scalar_tensor_tensor(
                        out=nxt,
                        in0=tree[:, :, :1],
                        scalar=RESCALE_K,
                        in1=tree[:, :, 1:],
                        op0=Mult,
                        op1=Mult,
                    )
                else:
                    nc.vector.tensor_mul(
                        out=nxt, in0=tree[:, :, : w // 2], in1=tree[:, :, w // 2 :]
                    )
                tree = nxt
                w //= 2
            lg = singles.tile([P, NB], f32)
            nc.scalar.activation(
                out=lg, in_=tree.rearrange("p c one -> p (c one)"), func=AF.Ln
            )
            lgT_ps = tp_ps.tile([NBT * NCH, P], f32, tag="tp")
            nc.tensor.transpose(lgT_ps, lg, ident)
            lgT = singles.tile([NBT * NCH, P], f32)
            nc.vector.tensor_scalar_add(lgT, lgT_ps, -RESCALE_LOG)
            nc.sync.dma_start(
                logdet.rearrange("(c p) -> c p", p=P), lgT
            )

    nc.finalize()
    return nc


def kernel(x, W1, b1, W2, b2):
    global LAST_RESULTS
    x = np.ascontiguousarray(x, dtype=np.float32)
    b1 = np.ascontiguousarray(b1, dtype=np.float32)
    b2 = np.ascontiguousarray(b2, dtype=np.float32)
    if MM_DTYPE == "float16":
        W1 = np.ascontiguousarray(W1, dtype=np.float16)
        W2 = np.ascontiguousarray(W2, dtype=np.float16)
    else:
        W1 = _round_fp32r(W1)
        W2 = _round_fp32r(W2)

    with_b2 = bool(np.any(b2))
    key = (with_b2, MM_DTYPE)
    if key not in _cache:
        _cache[key] = _build(with_b2)
    nc = _cache[key]

    in_maps = [
        {
            "x": x[i * B : (i + 1) * B],
            "w1": W1,
            "b1": b1,
            "w2": W2,
            "b2": b2,
        }
        for i in range(N_CORES)
    ]
    res = run_bass_kernel_spmd(
        nc, in_maps, core_ids=list(range(N_CORES)), trace=TRACE, tmpdir=TRACE_DIR
    )
    LAST_RESULTS = res
    outputs = np.concatenate([r["out"] for r in res.results], axis=0)
    logabsdet = np.concatenate(
        [r["logdet"].reshape(-1) for r in res.results], axis=0
    )
    return outputs, logabsdet


# revision 29
# speedup vs baseline: 1.0178x; 1.0178x over previous
"""Affine coupling transform (RealNVP-style) on 8 Trainium2 NeuronCores.

Data parallel: batch dim sharded 8 ways, weights replicated. Per core:
  x_shard [4096, 1024] -> out [4096, 1024], logabsdet [4096]

Math (per row):
  x_id = x[even cols], x_tr = x[odd cols]
  h = relu(x_id @ W1 + b1); params = h @ W2 + b2
  scale = sigmoid(params[:, :512] + 2) + 1e-3; shift = params[:, 512:]
  out[even] = x_id; out[odd] = x_tr * scale + shift
  logabsdet = sum(log(scale))

Kernel structure per core (b-tiles of 512 rows):
  - PE transposes x_id chunks to feature-major (fp32 DMA transpose unsupported)
  - both matmuls in float32r (full PE rate at N=512, fp32 data)
  - logabsdet via running product: prod((scale)*C) on DVE, single Ln at the
    end -- avoids per-tile ACT table swaps (sigmoid and ln live in different
    ACT table sets; each swap costs ~2.7us)
"""

import sys

if "/opt/trn_rl_repo" not in sys.path:
    sys.path.insert(0, "/opt/trn_rl_repo")

import math

import numpy as np

import concourse.bass as bass
import concourse.mybir as mybir
import concourse.tile as tile
from concourse import bacc
from concourse.bass_utils import run_bass_kernel_spmd
from concourse.masks import make_identity

N_CORES = 8
B_FULL = 32768
F_TOTAL = 1024
F_HALF = 512
H = 512
B = B_FULL // N_CORES  # rows per core
P = 128
BT = 512               # rows per b-tile (matmul moving dim)
NBT = B // BT          # b-tiles per core
NCH = BT // P          # 128-row chunks per b-tile
FC = F_HALF // P       # feature chunks (contraction of MM1)
JC = H // P            # hidden chunks (contraction of MM2)
SCALE_EPS = 1e-3
# The full 512-factor product of scales sits near exp(-77+-13); rescale by
# exp(+RESCALE_LOG) at the last tree level so the Ln input is centered near
# 1.0 (the HW Ln spline mishandles inputs far below 1).
RESCALE_LOG = 80.0
RESCALE_K = float(np.exp(RESCALE_LOG, dtype=np.float64))
# matmul operand dtype: "float16" (1 cyc/row on PE) or "float32r"
# (11-bit-mantissa fp32, but 2 cyc/row and serialized weight loads)
MM_DTYPE = "float16"
# "pe": tensor-engine transpose via identity matmul. ("dma" XBAR transpose
# hard-crashes the device: this Tile version has no xbar_mode hazard
# serialization, hitting the known DMATranspose/DMACopy HW hang.)
TRANSPOSE_MODE = "pe"

TRACE = False
TRACE_DIR = None
LAST_RESULTS = None

_cache = {}


def _round_fp32r(a: np.ndarray) -> np.ndarray:
    """Round fp32 to the fp32r memory format: mantissa rounded (RNE) to 11
    explicit bits, low 12 bits zero."""
    bits = np.ascontiguousarray(a, dtype=np.float32).view(np.uint32)
    keep = np.uint32(0xFFFFF000)
    half = np.uint32(0x7FF)
    lsb = (bits >> np.uint32(12)) & np.uint32(1)
    rounded = (bits + half + lsb) & keep
    return rounded.view(np.float32)

f32 = mybir.dt.float32
f32r = mybir.dt.float32r


def _build(with_b2: bool, mm_dtype=None) -> bass.Bass:
    fmm = {"float16": mybir.dt.float16, "float32r": f32r}[mm_dtype or MM_DTYPE]
    dma_tp = TRANSPOSE_MODE == "dma" and fmm == mybir.dt.float16
    nc = bacc.Bacc(None)

    x = nc.dram_tensor("x", [B, F_TOTAL], f32, kind="ExternalInput")
    w1 = nc.dram_tensor("w1", [F_HALF, H], fmm, kind="ExternalInput")
    b1 = nc.dram_tensor("b1", [H], f32, kind="ExternalInput")
    w2 = nc.dram_tensor("w2", [H, 2 * F_HALF], fmm, kind="ExternalInput")
    b2 = nc.dram_tensor("b2", [2 * F_HALF], f32, kind="ExternalInput")
    out = nc.dram_tensor("out", [B, F_TOTAL], f32, kind="ExternalOutput")
    logdet = nc.dram_tensor("logdet", [B], f32, kind="ExternalOutput")

    Mult = mybir.AluOpType.mult
    Add = mybir.AluOpType.add
    AF = mybir.ActivationFunctionType

    with tile.TileContext(nc) as tc:
        with (
            tc.tile_pool(name="singles", bufs=1) as singles,
            tc.tile_pool(name="xin", bufs=4) as xin,
            tc.tile_pool(name="xT", bufs=2) as xTp,
            tc.tile_pool(name="hT", bufs=2) as hTp,
            tc.tile_pool(name="eps", bufs=3) as epsp,
            tc.tile_pool(name="xc", bufs=3) as xcp,
            tc.tile_pool(name="tp_ps", bufs=(1 if dma_tp else 3), space="PSUM") as tp_ps,
            tc.tile_pool(name="h_ps", bufs=(3 if dma_tp else 2), space="PSUM") as h_ps,
            tc.tile_pool(name="p2_ps", bufs=(4 if dma_tp else 3), space="PSUM") as p2_ps,
        ):
            # ---- constants ----
            ident = singles.tile([P, P], f32)
            make_identity(nc, ident)

            w1s = singles.tile([P, FC, H], fmm)
            w2s = singles.tile([P, JC, 2 * F_HALF], fmm)
            b1s = singles.tile([P, JC], f32)
            nc.sync.dma_start(b1s, b1.rearrange("(jo ji) -> ji jo", ji=P))
            if with_b2:
                b2s = singles.tile([P, 2 * F_HALF], f32)
                nc.sync.dma_start(
                    b2s, b2[None, :].broadcast_to([P, 2 * F_HALF])
                )
            two_col = singles.tile([P, 1], f32)
            nc.vector.memset(two_col, 2.0)

            # per-row partial products of scale*C: block t*NCH+c holds 128
            # 4-factor partials for rows [ (t*NCH+c)*128 , +128 )
            prodacc = singles.tile([P, NBT * NCH, P // 2], f32)

            def load_tile(t):
                r0 = t * BT
                xb = xin.tile([P, NCH, F_TOTAL], f32, tag="xt")
                if t == 0:
                    # per-chunk DMAs so the first transpose isn't gated on
                    # the whole 2MB tile landing
                    for c in range(NCH):
                        nc.sync.dma_start(
                            xb[:, c, :], x[r0 + c * P : r0 + (c + 1) * P, :]
                        )
                else:
                    nc.sync.dma_start(
                        xb,
                        x[r0 : r0 + BT, :].rearrange("(c p) f -> p c f", p=P),
                    )
                return xb, [xb[:, c, :] for c in range(NCH)]

            def transpose_tile(xts):
                xT = xTp.tile([P, FC, BT], fmm, tag="xT")
                for c in range(NCH):
                    xtr = xts[c].rearrange("p (f two) -> p f two", two=2)
                    if dma_tp:
                        # cast even cols to fp16, then XBAR-transpose via DMA
                        xc = xcp.tile([P, F_HALF], fmm, tag="xc")
                        nc.vector.tensor_copy(out=xc, in_=xtr[:, :, 0])
                        for fc in range(FC):
                            nc.sync.dma_start_transpose(
                                xT[:, fc, c * P : (c + 1) * P],
                                xc[:, fc * P : (fc + 1) * P],
                            )
                    else:
                        for fc in range(FC):
                            tps = tp_ps.tile([P, P], f32, tag="tp")
                            nc.tensor.transpose(
                                tps, xtr[:, fc * P : (fc + 1) * P, 0], ident
                            )
                            nc.vector.tensor_copy(
                                out=xT[:, fc, c * P : (c + 1) * P], in_=tps
                            )
                return xT

            # software-pipelined emission: the next tile's transposes (PE) and
            # their PSUM->SBUF copies (DVE) are queued BEFORE this tile's
            # MM2/epilogue, so the strict-FIFO DVE queue doesn't head-of-line
            # block the next tile's transposes behind the epilogue.
            xb, xts = load_tile(0)
            nc.sync.dma_start(w1s, w1.rearrange("(fo fi) j -> fi fo j", fi=P))
            xT = transpose_tile(xts)
            nc.sync.dma_start(w2s, w2.rearrange("(jo ji) p -> ji jo p", ji=P))
            next_xts = None
            next_xT = None
            for t in range(NBT):
                r0 = t * BT
                if t + 1 < NBT:
                    next_xb, next_xts = load_tile(t + 1)

                # ---- MM1: hT[j, b] = relu(sum_f W1[f,j] xT[f,b] + b1[j]) ----
                hT = hTp.tile([P, JC, BT], fmm, tag="hT")
                for jc in range(JC):
                    hps = h_ps.tile([P, BT], f32, tag="h")
                    for fc in range(FC):
                        nc.tensor.matmul(
                            hps,
                            w1s[:, fc, jc * P : (jc + 1) * P],
                            xT[:, fc, :],
                            start=(fc == 0),
                            stop=(fc == FC - 1),
                        )
                    nc.scalar.activation(
                        out=hT[:, jc, :],
                        in_=hps,
                        func=AF.Relu,
                        bias=b1s[:, jc : jc + 1],
                    )

                if t + 1 < NBT:
                    next_xT = transpose_tile(next_xts)

                # ---- MM2 + coupling epilogue, per 128-row chunk ----
                for c in range(NCH):
                    pps = []
                    for ph in range(2):
                        pp = p2_ps.tile([P, F_HALF], f32, tag="p2")
                        for jc in range(JC):
                            nc.tensor.matmul(
                                pp,
                                hT[:, jc, c * P : (c + 1) * P],
                                w2s[:, jc, ph * F_HALF : (ph + 1) * F_HALF],
                                start=(jc == 0),
                                stop=(jc == JC - 1),
                            )
                        pps.append(pp)

                    if with_b2:
                        nc.vector.tensor_add(
                            out=pps[0], in0=pps[0], in1=b2s[:, :F_HALF]
                        )
                        nc.vector.tensor_add(
                            out=pps[1], in0=pps[1], in1=b2s[:, F_HALF:]
                        )

                    # scale path: sigmoid(u + 2), then cs = (sig + eps)*C;
                    # two pairwise-product halvings leave 128 partials per
                    # row in prodacc (reduced to one product at the end)
                    sig = epsp.tile([P, F_HALF], f32, tag="sig")
                    nc.scalar.activation(
                        out=sig, in_=pps[0], func=AF.Sigmoid, bias=two_col
                    )
                    cs = epsp.tile([P, F_HALF], f32, tag="cs")
                    nc.vector.tensor_scalar_add(cs, sig, SCALE_EPS)
                    idx = t * NCH + c
                    q1 = epsp.tile([P, F_HALF // 2], f32, tag="q1")
                    nc.vector.tensor_mul(
                        out=q1, in0=cs[:, : F_HALF // 2], in1=cs[:, F_HALF // 2 :]
                    )
                    q2 = epsp.tile([P, F_HALF // 4], f32, tag="q2")
                    nc.vector.tensor_mul(out=q2, in0=q1[:, :P], in1=q1[:, P:])
                    nc.vector.tensor_mul(
                        out=prodacc[:, idx, :],
                        in0=q2[:, : P // 2],
                        in1=q2[:, P // 2 :],
                    )

                    # out[odd] = x_tr * (cs/C) + shift ; out[even] = x_id
                    # out[even] = x[even] for free: x was loaded into this
                    # buffer and only the odd columns are overwritten
                    xtr = xts[c].rearrange("p (f two) -> p f two", two=2)
                    tmp = epsp.tile([P, F_HALF], f32, tag="tmp")
                    nc.vector.tensor_mul(out=tmp, in0=xtr[:, :, 1], in1=cs)
                    nc.vector.tensor_add(out=xtr[:, :, 1], in0=tmp, in1=pps[1])

                    if t == NBT - 1:
                        nc.sync.dma_start(
                            out[r0 + c * P : r0 + (c + 1) * P, :], xts[c]
                        )
                    elif c == NCH - 1:
                        nc.sync.dma_start(
                            out[r0 : r0 + BT, :].rearrange(
                                "(c p) f -> p c f", p=P
                            ),
                            xb,
                        )

                xb = next_xb
                xts = next_xts
                xT = next_xT

            # ---- reduce the 128 partials per block to one product via an
            # in-block pairwise tree, then logabsdet = ln(prod) - 512*ln(C),
            # transposed for a contiguous DRAM write ----
            NB = NBT * NCH
            tree = prodacc
            w = P // 2
            while w > 1:
                nxt = singles.tile([P, NB, w // 2], f32, tag=f"tree{w}")
                if w == 2:
                    # last level: fold in exp(+RESCALE_LOG) to keep the full
                    # 512-factor product inside fp32 range
                    nc.vector.scalar_tensor_tensor(
                        out=nxt,
                        in0=tree[:, :, :1],
                        scalar=RESCALE_K,
                        in1=tree[:, :, 1:],
                        op0=Mult,
                        op1=Mult,
                    )
                else:
                    nc.vector.tensor_mul(
                        out=nxt, in0=tree[:, :, : w // 2], in1=tree[:, :, w // 2 :]
                    )
                tree = nxt
                w //= 2
            lg = singles.tile([P, NB], f32)
            nc.scalar.activation(
                out=lg, in_=tree.rearrange("p c one -> p (c one)"), func=AF.Ln
            )
            lgT_ps = tp_ps.tile([NBT * NCH, P], f32, tag="tp")
            nc.tensor.transpose(lgT_ps, lg, ident)
            lgT = singles.tile([NBT * NCH, P], f32)
            nc.vector.tensor_scalar_add(lgT, lgT_ps, -RESCALE_LOG)
            nc.sync.dma_start(
                logdet.rearrange("(c p) -> c p", p=P), lgT
            )

    nc.finalize()
    return nc


def kernel(x, W1, b1, W2, b2):
    global LAST_RESULTS
    x = np.ascontiguousarray(x, dtype=np.float32)
    b1 = np.ascontiguousarray(b1, dtype=np.float32)
    b2 = np.ascontiguousarray(b2, dtype=np.float32)
    if MM_DTYPE == "float16":
        W1 = np.ascontiguousarray(W1, dtype=np.float16)
        W2 = np.ascontiguousarray(W2, dtype=np.float16)
    else:
        W1 = _round_fp32r(W1)
        W2 = _round_fp32r(W2)

    with_b2 = bool(np.any(b2))
    key = (with_b2, MM_DTYPE)
    if key not in _cache:
        _cache[key] = _build(with_b2)
    nc = _cache[key]

    in_maps = [
        {
            "x": x[i * B : (i + 1) * B],
            "w1": W1,
            "b1": b1,
            "w2": W2,
            "b2": b2,
        }
        for i in range(N_CORES)
    ]
    res = run_bass_kernel_spmd(
        nc, in_maps, core_ids=list(range(N_CORES)), trace=TRACE, tmpdir=TRACE_DIR
    )
    LAST_RESULTS = res
    outputs = np.concatenate([r["out"] for r in res.results], axis=0)
    logabsdet = np.concatenate(
        [r["logdet"].reshape(-1) for r in res.results], axis=0
    )
    return outputs, logabsdet
